# revision 10
# baseline (speedup 1.0000x reference)
"""Expert-parallel MoE layer for Trainium2 (8 NeuronCores, one expert per core).

Host side (numpy): router logits, exact top-2 dispatch, p0 weights, and the
scatter-add combine. Device side (Bass/Tile, SPMD over 8 cores): the dense FFN
y = gelu(x @ W1[e] + b1[e]) @ W2[e] over the tokens routed to expert e,
computed with fp16 operands (fp32 PSUM accumulation).

Per-core layout: F (the 4096-wide hidden dim) is processed in NQ=4 quarters
with W1/W2 quarter-slices streamed through SBUF (double-buffered); the whole
fp16 xT stays resident. GEMM2 is computed transposed (stationary = W2 chunk,
moving = hT) so its PE cost scales with the exact token count instead of
quantizing to 128-row tiles, and y^T accumulates across quarters in SBUF via
DVE adds (no DRAM read-modify-write traffic, near-zero drain tail: each token
group's y is DMA'd out as soon as its last-quarter add lands).
"""

import numpy as np

B, S, H, E, F = 4, 2048, 1024, 8, 4096
T = B * S
P = 128
NQ = 4              # F quarters (outer loop); W1q + W2q streamed per quarter
FQ = F // NQ
KH = H // P         # k-chunks over H (GEMM1 contraction)
KFQ = FQ // P       # k-chunks over one F quarter (GEMM2 contraction)
HC = H // P         # output H chunks (GEMM2 transposed: psum partition = h')
TT = 512            # token group (matmul moving free dim)
MIN_CAP = 64

_cache = {}


def _spill_waits(nc, mybir, max_waits=1):
    """walrus CoreV2/V3 codegen rejects instructions with >1 semaphore wait
    ("Too many sync wait commands") — notably self-loading fp32/fp32r matmuls
    and DMACopy. Move excess waits onto same-engine no-ops inserted right
    before the instruction (sequencers run in order, so this is equivalent)."""
    for fn in nc.m.functions:
        for blk in fn.blocks:
            out = []
            changed = False
            for inst in blk.instructions:
                si = getattr(inst, "sync_info", None)
                if si is not None and len(si.on_wait) > max_waits:
                    spill = si.on_wait[: len(si.on_wait) - max_waits]
                    keep = si.on_wait[len(si.on_wait) - max_waits:]
                    for w in spill:
                        nop = mybir.InstNoOp(
                            name=nc.get_next_instruction_name(),
                            engine=inst.engine,
                            ins=[],
                            outs=[],
                        )
                        nop.sync_info = mybir.SyncInfo(on_wait=[w], on_update=[])
                        out.append(nop)
                    inst.sync_info = mybir.SyncInfo(on_wait=keep, on_update=si.on_update)
                    changed = True
                out.append(inst)
            if changed:
                blk.instructions = out


def _build(cap):
    import concourse.bass as bass
    import concourse.mybir as mybir
    from concourse import tile

    F32 = mybir.dt.float32
    # all matmul operands bf16: same PE speed as fp16, but the 7-bit mantissa
    # toggles far less of the multiplier array, which keeps the PE further
    # from its power-throttle ceiling (fp16 measurably throttles harder)
    SDT = mybir.dt.bfloat16
    GELU = mybir.ActivationFunctionType.Gelu_apprx_tanh
    ADD = mybir.AluOpType.add

    nc = bass.Bass()
    xt = nc.declare_dram_parameter("xt", [H, cap], SDT, isOutput=False)
    w1 = nc.declare_dram_parameter("w1", [H, F], SDT, isOutput=False)
    w2 = nc.declare_dram_parameter("w2", [F, H], SDT, isOutput=False)
    b1s = nc.declare_dram_parameter("b1s", [P, F // P], F32, isOutput=False)
    yt = nc.declare_dram_parameter("yt", [H, cap], F32, isOutput=True)

    sizes = []
    o = 0
    while o < cap:
        tt = min(TT, cap - o)
        sizes.append(tt)
        o += tt
    if len(sizes) >= 2 and sizes[-1] < 280:
        # a tiny remainder group is LDWEIGHTS-bound (stream < weight-load);
        # split the last two groups evenly so both stay above the crossover
        pair = sizes[-2] + sizes[-1]
        sizes[-2] = -(-pair // 2)
        sizes[-1] = pair - sizes[-2]
    groups = []
    o = 0
    for tt in sizes:
        groups.append((o, tt))
        o += tt

    with tile.TileContext(nc) as tc:
        with (
            tc.tile_pool(name="w1p", bufs=2) as w1p,
            tc.tile_pool(name="w2p", bufs=2) as w2p,
            tc.tile_pool(name="xp", bufs=1) as xp,
            tc.tile_pool(name="hp", bufs=1) as hp,
            tc.tile_pool(name="yp", bufs=1) as yp,
            tc.tile_pool(name="cst", bufs=1) as cst,
            tc.tile_pool(name="ps1", bufs=4, space="PSUM") as ps1,
            tc.tile_pool(name="ps2", bufs=4, space="PSUM") as ps2,
        ):
            def load_w1(q):
                w1q = w1p.tile([P, KH, FQ], SDT, tag="w1q")
                src = w1[:, q * FQ:(q + 1) * FQ].rearrange("(c p) f -> p c f", p=P)
                nc.sync.dma_start(w1q[:], src)
                return w1q

            def load_w2(q, split=False):
                # DMA issues ride gpsimd/sync so the Scalar engine (which runs
                # the critical-path activations) never stalls on a descriptor
                w2q = w2p.tile([P, KFQ, H], SDT, tag="w2q")
                src = w2[q * FQ:(q + 1) * FQ, :].rearrange("(c p) h -> p c h", p=P)
                if split:
                    nc.gpsimd.dma_start(w2q[:, : KFQ // 2, :], src[:, : KFQ // 2, :])
                    nc.sync.dma_start(w2q[:, KFQ // 2:, :], src[:, KFQ // 2:, :])
                else:
                    nc.gpsimd.dma_start(w2q[:], src)
                return w2q

            # prologue: interleave x(group 0) and W1(quarter 0) per k-chunk
            # across four engine DMA queues (each sustains only ~80-100GB/s)
            # so GEMM1's first accumulation chain can start after one
            # chunk-pair (~400KB) instead of the full ~3MB; the rest of x
            # (resident for the whole kernel) follows on gpsimd
            b1t = cst.tile([P, F // P], F32)
            nc.scalar.dma_start(b1t[:], b1s[:])
            x_all = xp.tile([P, KH, cap], SDT)
            xsrc = xt.rearrange("(c p) t -> p c t", p=P)
            g0 = groups[0][1]
            w1q = w1p.tile([P, KH, FQ], SDT, tag="w1q")
            w1src = w1[:, :FQ].rearrange("(c p) f -> p c f", p=P)
            for k in range(KH):
                wq = nc.sync if k % 2 == 0 else nc.gpsimd
                nc.scalar.dma_start(x_all[:, k, :g0], xsrc[:, k, :g0])
                wq.dma_start(w1q[:, k, :], w1src[:, k, :])
            if cap > g0:
                nc.gpsimd.dma_start(x_all[:, :, g0:], xsrc[:, :, g0:])
            yT = yp.tile([P, HC, cap], F32)
            ytdram = yt.rearrange("(c p) t -> p c t", p=P)
            w2q = None
            for q in range(NQ):
                for gi, (t0, tt) in enumerate(groups):
                    # GEMM1: hT[f, t] = sum_h W1[h, f] * xT[h, t], then gelu
                    hq = hp.tile([P, KFQ, TT], SDT, tag="hq")
                    for fs in range(KFQ):
                        pt = ps1.tile([P, TT], F32, tag="pt1")
                        for k in range(KH):
                            nc.tensor.matmul(
                                pt[:, :tt],
                                w1q[:, k, fs * P:(fs + 1) * P],
                                x_all[:, k, t0:t0 + tt],
                                start=(k == 0),
                                stop=(k == KH - 1),
                            )
                        c = q * KFQ + fs
                        nc.scalar.activation(
                            hq[:, fs, :tt], pt[:, :tt], GELU, bias=b1t[:, c:c + 1]
                        )
                    if q == 0 and gi == 0:
                        # W2 deliberately after GEMM1(group 0): its first use
                        # is GEMM2, so don't let it contend with W1/x at start
                        w2q = load_w2(0, split=True)
                    if gi == 0 and q + 1 < NQ:
                        w1_nxt = load_w1(q + 1)
                    if gi == 2 and q + 1 < NQ:
                        w2_nxt = load_w2(q + 1)
                    # GEMM2 (transposed): yT[h', t] += sum_f W2[f, h'] * hT[f, t]
                    for hc in range(HC):
                        pt2 = ps2.tile([P, TT], F32, tag="pt2")
                        for k2 in range(KFQ):
                            nc.tensor.matmul(
                                pt2[:, :tt],
                                w2q[:, k2, hc * P:(hc + 1) * P],
                                hq[:, k2, :tt],
                                start=(k2 == 0),
                                stop=(k2 == KFQ - 1),
                            )
                        if q == 0:
                            nc.vector.tensor_copy(
                                yT[:, hc, t0:t0 + tt], pt2[:, :tt]
                            )
                        else:
                            nc.vector.tensor_tensor(
                                yT[:, hc, t0:t0 + tt],
                                yT[:, hc, t0:t0 + tt],
                                pt2[:, :tt],
                                ADD,
                            )
                        if q == NQ - 1:
                            # y^T for this (group, hc) is final: stream it out
                            # as soon as its add lands so almost nothing
                            # drains after the last matmul; alternate queues
                            # so a single backlogged queue can't delay the end
                            yq = nc.sync if hc % 2 == 0 else nc.gpsimd
                            yq.dma_start(
                                ytdram[:, hc, t0:t0 + tt],
                                yT[:, hc, t0:t0 + tt],
                            )
                if q + 1 < NQ:
                    w1q, w2q = w1_nxt, w2_nxt

    import concourse.mybir as mybir_mod

    _spill_waits(nc, mybir_mod)
    return nc


def _route(x2d, Wr, br):
    """Top-2 routing, bit-matching the reference's decisions.

    Softmax is monotonic, so top-2-of-probs == top-2-of-logits, and the
    normalized top-1 weight p0 = p1/(p1+p2) == sigmoid(l1-l2) exactly (the
    softmax denominator cancels). Ordering ties are broken by lower index,
    same as jax.lax.top_k."""
    logits = x2d @ np.asarray(Wr, np.float32) + np.asarray(br, np.float32)
    order = np.argsort(-logits, axis=-1, kind="stable")
    i1 = order[:, 0].astype(np.int64)
    i2 = order[:, 1].astype(np.int64)
    r = np.arange(logits.shape[0])
    l1 = logits[r, i1].astype(np.float64)
    l2 = logits[r, i2].astype(np.float64)
    p0 = 1.0 / (1.0 + np.exp(l2 - l1))
    return i1, i2, p0.astype(np.float32)


def _plan(x, Wr, br, W1, b1, W2, b2):
    """Route on host; build the per-core (per-expert) input maps."""
    x2d = np.ascontiguousarray(np.asarray(x, np.float32).reshape(T, H))
    W1 = np.asarray(W1, np.float32)
    b1 = np.asarray(b1, np.float32)
    W2 = np.asarray(W2, np.float32)

    i1, i2, p0 = _route(x2d, Wr, br)
    idxs = [np.flatnonzero((i1 == e) | (i2 == e)) for e in range(E)]
    max_cnt = max(len(ix) for ix in idxs)
    cap = max(MIN_CAP, -(-max_cnt // 2) * 2)

    import ml_dtypes

    BF16 = ml_dtypes.bfloat16
    xT = np.ascontiguousarray(x2d.T)  # [H, T]
    in_maps = []
    for e in range(E):
        ix = idxs[e]
        xte = np.zeros((H, cap), np.float32)
        xte[:, : len(ix)] = xT[:, ix]
        b1se = np.ascontiguousarray(b1[e].reshape(F // P, P).T)
        in_maps.append(
            {
                "xt": xte.astype(BF16),
                "w1": np.ascontiguousarray(W1[e]).astype(BF16),
                "w2": np.ascontiguousarray(W2[e]).astype(BF16),
                "b1s": b1se,
            }
        )
    return cap, in_maps, idxs, p0


def kernel(x, Wr, br, W1, b1, W2, b2):
    from concourse.bass_utils import run_bass_kernel_spmd

    cap, in_maps, idxs, p0 = _plan(x, Wr, br, W1, b1, W2, b2)

    if cap not in _cache:
        _cache[cap] = _build(cap)
    nc = _cache[cap]

    try:
        res = run_bass_kernel_spmd(nc, in_maps, list(range(E)))
    except Exception:
        import time as _time

        _time.sleep(10)
        res = run_bass_kernel_spmd(nc, in_maps, list(range(E)))

    b2 = np.asarray(b2, np.float32)
    out = np.zeros((T, H), np.float32)
    for e in range(E):
        ix = idxs[e]
        ye = res.results[e]["yt"][:, : len(ix)].T  # [cnt, H]
        out[ix] += p0[ix, None] * (ye + b2[e][None, :])
    return out.reshape(B, S, H)


# revision 13
# speedup vs baseline: 1.0096x; 1.0096x over previous
"""Expert-parallel MoE layer for Trainium2 (8 NeuronCores, one expert per core).

Host side (numpy): router logits, exact top-2 dispatch, p0 weights, and the
scatter-add combine. Device side (Bass/Tile, SPMD over 8 cores): the dense FFN
y = gelu(x @ W1[e] + b1[e]) @ W2[e] over the tokens routed to expert e,
computed with fp16 operands (fp32 PSUM accumulation).

Per-core layout: F (the 4096-wide hidden dim) is processed in NQ=4 quarters
with W1/W2 quarter-slices streamed through SBUF (double-buffered); the whole
fp16 xT stays resident. GEMM2 is computed transposed (stationary = W2 chunk,
moving = hT) so its PE cost scales with the exact token count instead of
quantizing to 128-row tiles, and y^T accumulates across quarters in SBUF via
DVE adds (no DRAM read-modify-write traffic, near-zero drain tail: each token
group's y is DMA'd out as soon as its last-quarter add lands).
"""

import numpy as np

B, S, H, E, F = 4, 2048, 1024, 8, 4096
T = B * S
P = 128
NQ = 4              # F quarters (outer loop); W1q + W2q streamed per quarter
FQ = F // NQ
KH = H // P         # k-chunks over H (GEMM1 contraction)
KFQ = FQ // P       # k-chunks over one F quarter (GEMM2 contraction)
HC = H // P         # output H chunks (GEMM2 transposed: psum partition = h')
TT = 512            # token group (matmul moving free dim)
MIN_CAP = 64

_cache = {}


def _spill_waits(nc, mybir, max_waits=1):
    """walrus CoreV2/V3 codegen rejects instructions with >1 semaphore wait
    ("Too many sync wait commands") — notably self-loading fp32/fp32r matmuls
    and DMACopy. Move excess waits onto same-engine no-ops inserted right
    before the instruction (sequencers run in order, so this is equivalent)."""
    for fn in nc.m.functions:
        for blk in fn.blocks:
            out = []
            changed = False
            for inst in blk.instructions:
                si = getattr(inst, "sync_info", None)
                if si is not None and len(si.on_wait) > max_waits:
                    spill = si.on_wait[: len(si.on_wait) - max_waits]
                    keep = si.on_wait[len(si.on_wait) - max_waits:]
                    for w in spill:
                        nop = mybir.InstNoOp(
                            name=nc.get_next_instruction_name(),
                            engine=inst.engine,
                            ins=[],
                            outs=[],
                        )
                        nop.sync_info = mybir.SyncInfo(on_wait=[w], on_update=[])
                        out.append(nop)
                    inst.sync_info = mybir.SyncInfo(on_wait=keep, on_update=si.on_update)
                    changed = True
                out.append(inst)
            if changed:
                blk.instructions = out


def _groups(cap):
    """Token groups: a small first group (so the startup DMA for it is tiny
    and the PE starts early), 512s after, and no tiny remainder group (tiny
    groups are LDWEIGHTS-bound: stream time < weight-load time)."""
    sizes = []
    o = 0
    while o < cap:
        tt = min(256 if not sizes else TT, cap - o)
        sizes.append(tt)
        o += tt
    if len(sizes) >= 2 and sizes[-1] < 280:
        pair = sizes[-2] + sizes[-1]
        sizes[-2] = -(-pair // 2)
        sizes[-1] = pair - sizes[-2]
    groups = []
    o = 0
    for tt in sizes:
        groups.append((o, tt))
        o += tt
    return groups


def _build(cap):
    import concourse.bass as bass
    import concourse.mybir as mybir
    from concourse import tile

    F32 = mybir.dt.float32
    # all matmul operands bf16: same PE speed as fp16, but the 7-bit mantissa
    # toggles far less of the multiplier array, which keeps the PE further
    # from its power-throttle ceiling (fp16 measurably throttles harder)
    SDT = mybir.dt.bfloat16
    GELU = mybir.ActivationFunctionType.Gelu_apprx_tanh
    ADD = mybir.AluOpType.add

    groups = _groups(cap)

    # all inputs are host-packed in exact SBUF layout, so every DMA row is
    # one long contiguous run per partition (DMA throughput is dominated by
    # per-row overhead; 1KB rows move ~3x slower than 8KB rows):
    #   xt: group-major, block g = [KH, tt] per partition
    #   w1: [q][fs][k][128] per partition (fs-major: the startup quarter
    #       streams in 8 fs-blocks, each immediately usable by one GEMM1
    #       accumulation chain)
    #   w2: [q][k2][H] per partition
    nc = bass.Bass()
    xt = nc.declare_dram_parameter("xt", [P, KH * cap], SDT, isOutput=False)
    w1 = nc.declare_dram_parameter("w1", [P, NQ * KFQ * KH * P], SDT, isOutput=False)
    w2 = nc.declare_dram_parameter("w2", [P, NQ * KFQ * H], SDT, isOutput=False)
    b1s = nc.declare_dram_parameter("b1s", [P, F // P], F32, isOutput=False)
    yt = nc.declare_dram_parameter("yt", [H, cap], F32, isOutput=True)

    with tile.TileContext(nc) as tc:
        with (
            tc.tile_pool(name="w1p", bufs=2) as w1p,
            tc.tile_pool(name="w2p", bufs=2) as w2p,
            tc.tile_pool(name="xp", bufs=1) as xp,
            tc.tile_pool(name="hp", bufs=1) as hp,
            tc.tile_pool(name="yp", bufs=1) as yp,
            tc.tile_pool(name="cst", bufs=1) as cst,
            tc.tile_pool(name="ps1", bufs=4, space="PSUM") as ps1,
            tc.tile_pool(name="ps2", bufs=4, space="PSUM") as ps2,
        ):
            def load_w1(q):
                # steady-state quarters: one DMA, 16KB contiguous rows
                w1q = w1p.tile([P, KFQ, KH, P], SDT, tag="w1q")
                nc.sync.dma_start(w1q[:], w1[:, q * QW1:(q + 1) * QW1])
                return w1q

            def load_w2(q, split=False):
                # DMA issues ride gpsimd/sync so the Scalar engine (which runs
                # the critical-path activations) never stalls on a descriptor
                w2q = w2p.tile([P, KFQ, H], SDT, tag="w2q")
                src = w2[:, q * QW2:(q + 1) * QW2]
                if split:
                    nc.gpsimd.dma_start(w2q[:, : KFQ // 2, :], src[:, : QW2 // 2])
                    nc.sync.dma_start(w2q[:, KFQ // 2:, :], src[:, QW2 // 2:])
                else:
                    nc.gpsimd.dma_start(w2q[:], src)
                return w2q

            QW1 = KFQ * KH * P
            QW2 = KFQ * H
            xoff = []
            o = 0
            for _, tt in groups:
                xoff.append(o)
                o += KH * tt

            # prologue. DMA throughput is row-overhead-bound, so feed order
            # matters more than anything: x(group 0) is one small DMA on
            # scalar, W1(quarter 0) streams as 8 fs-blocks (2KB rows) split
            # over sync+gpsimd so GEMM1 chains start after ~2 blocks; the
            # remaining x groups follow on scalar, W2(quarter 0) on gpsimd.
            b1t = cst.tile([P, F // P], F32)
            nc.scalar.dma_start(b1t[:], b1s[:])
            x_all = xp.tile([P, KH * cap], SDT)
            nc.scalar.dma_start(x_all[:, : KH * groups[0][1]],
                                xt[:, : KH * groups[0][1]])
            w1q = w1p.tile([P, KFQ, KH, P], SDT, tag="w1q")
            BL = KH * P
            for fs in range(KFQ):
                wq = nc.sync if fs % 2 == 0 else nc.gpsimd
                wq.dma_start(w1q[:, fs, :, :], w1[:, fs * BL:(fs + 1) * BL])
            for gi in range(1, len(groups)):
                lo, hi = xoff[gi], xoff[gi] + KH * groups[gi][1]
                nc.scalar.dma_start(x_all[:, lo:hi], xt[:, lo:hi])
            yT = yp.tile([P, HC, cap], F32)
            ytdram = yt.rearrange("(c p) t -> p c t", p=P)
            w2q = load_w2(0)
            for q in range(NQ):
                for gi, (t0, tt) in enumerate(groups):
                    xo = xoff[gi]
                    # GEMM1: hT[f, t] = sum_h W1[h, f] * xT[h, t], then gelu
                    hq = hp.tile([P, KFQ, TT], SDT, tag="hq")
                    for fs in range(KFQ):
                        pt = ps1.tile([P, TT], F32, tag="pt1")
                        for k in range(KH):
                            nc.tensor.matmul(
                                pt[:, :tt],
                                w1q[:, fs, k, :],
                                x_all[:, xo + k * tt: xo + (k + 1) * tt],
                                start=(k == 0),
                                stop=(k == KH - 1),
                            )
                        c = q * KFQ + fs
                        nc.scalar.activation(
                            hq[:, fs, :tt], pt[:, :tt], GELU, bias=b1t[:, c:c + 1]
                        )
                    if gi == 0 and q + 1 < NQ:
                        w1_nxt = load_w1(q + 1)
                    if gi == 2 and q + 1 < NQ:
                        w2_nxt = load_w2(q + 1)
                    # GEMM2 (transposed): yT[h', t] += sum_f W2[f, h'] * hT[f, t]
                    for hc in range(HC):
                        pt2 = ps2.tile([P, TT], F32, tag="pt2")
                        for k2 in range(KFQ):
                            nc.tensor.matmul(
                                pt2[:, :tt],
                                w2q[:, k2, hc * P:(hc + 1) * P],
                                hq[:, k2, :tt],
                                start=(k2 == 0),
                                stop=(k2 == KFQ - 1),
                            )
                        if q == 0:
                            nc.vector.tensor_copy(
                                yT[:, hc, t0:t0 + tt], pt2[:, :tt]
                            )
                        else:
                            nc.vector.tensor_tensor(
                                yT[:, hc, t0:t0 + tt],
                                yT[:, hc, t0:t0 + tt],
                                pt2[:, :tt],
                                ADD,
                            )
                        if q == NQ - 1:
                            # y^T for this (group, hc) is final: stream it out
                            # as soon as its add lands so almost nothing
                            # drains after the last matmul; alternate queues
                            # so a single backlogged queue can't delay the end
                            yq = nc.sync if hc % 2 == 0 else nc.gpsimd
                            yq.dma_start(
                                ytdram[:, hc, t0:t0 + tt],
                                yT[:, hc, t0:t0 + tt],
                            )
                if q + 1 < NQ:
                    w1q, w2q = w1_nxt, w2_nxt

    import concourse.mybir as mybir_mod

    _spill_waits(nc, mybir_mod)
    return nc


def _route(x2d, Wr, br):
    """Top-2 routing, bit-matching the reference's decisions.

    Softmax is monotonic, so top-2-of-probs == top-2-of-logits, and the
    normalized top-1 weight p0 = p1/(p1+p2) == sigmoid(l1-l2) exactly (the
    softmax denominator cancels). Ordering ties are broken by lower index,
    same as jax.lax.top_k."""
    logits = x2d @ np.asarray(Wr, np.float32) + np.asarray(br, np.float32)
    order = np.argsort(-logits, axis=-1, kind="stable")
    i1 = order[:, 0].astype(np.int64)
    i2 = order[:, 1].astype(np.int64)
    r = np.arange(logits.shape[0])
    l1 = logits[r, i1].astype(np.float64)
    l2 = logits[r, i2].astype(np.float64)
    p0 = 1.0 / (1.0 + np.exp(l2 - l1))
    return i1, i2, p0.astype(np.float32)


def _plan(x, Wr, br, W1, b1, W2, b2):
    """Route on host; build the per-core (per-expert) input maps."""
    x2d = np.ascontiguousarray(np.asarray(x, np.float32).reshape(T, H))
    W1 = np.asarray(W1, np.float32)
    b1 = np.asarray(b1, np.float32)
    W2 = np.asarray(W2, np.float32)

    i1, i2, p0 = _route(x2d, Wr, br)
    idxs = [np.flatnonzero((i1 == e) | (i2 == e)) for e in range(E)]
    max_cnt = max(len(ix) for ix in idxs)
    cap = max(MIN_CAP, -(-max_cnt // 2) * 2)

    import ml_dtypes

    BF16 = ml_dtypes.bfloat16
    groups = _groups(cap)
    xT = np.ascontiguousarray(x2d.T)  # [H, T]
    in_maps = []
    for e in range(E):
        ix = idxs[e]
        xte = np.zeros((H, cap), np.float32)
        xte[:, : len(ix)] = xT[:, ix]
        # pack everything in exact SBUF layout (see _build)
        xr = xte.reshape(KH, P, cap)
        xpk = np.concatenate(
            [
                xr[:, :, t0:t0 + tt].transpose(1, 0, 2).reshape(P, KH * tt)
                for t0, tt in groups
            ],
            axis=1,
        )
        w1pk = (
            W1[e]
            .reshape(KH, P, NQ, KFQ, P)
            .transpose(1, 2, 3, 0, 4)
            .reshape(P, NQ * KFQ * KH * P)
        )
        w2pk = (
            W2[e]
            .reshape(NQ, KFQ, P, H)
            .transpose(2, 0, 1, 3)
            .reshape(P, NQ * KFQ * H)
        )
        b1se = np.ascontiguousarray(b1[e].reshape(F // P, P).T)
        in_maps.append(
            {
                "xt": np.ascontiguousarray(xpk).astype(BF16),
                "w1": np.ascontiguousarray(w1pk).astype(BF16),
                "w2": np.ascontiguousarray(w2pk).astype(BF16),
                "b1s": b1se,
            }
        )
    return cap, in_maps, idxs, p0


def kernel(x, Wr, br, W1, b1, W2, b2):
    from concourse.bass_utils import run_bass_kernel_spmd

    cap, in_maps, idxs, p0 = _plan(x, Wr, br, W1, b1, W2, b2)

    if cap not in _cache:
        _cache[cap] = _build(cap)
    nc = _cache[cap]

    try:
        res = run_bass_kernel_spmd(nc, in_maps, list(range(E)))
    except Exception:
        import time as _time

        _time.sleep(10)
        res = run_bass_kernel_spmd(nc, in_maps, list(range(E)))

    b2 = np.asarray(b2, np.float32)
    out = np.zeros((T, H), np.float32)
    for e in range(E):
        ix = idxs[e]
        ye = res.results[e]["yt"][:, : len(ix)].T  # [cnt, H]
        out[ix] += p0[ix, None] * (ye + b2[e][None, :])
    return out.reshape(B, S, H)


# revision 16
# speedup vs baseline: 1.0313x; 1.0215x over previous
"""Expert-parallel MoE layer for Trainium2 (8 NeuronCores, one expert per core).

Host side (numpy): router logits, exact top-2 dispatch, p0 weights, and the
scatter-add combine. Device side (Bass/Tile, SPMD over 8 cores): the dense FFN
y = gelu(x @ W1[e] + b1[e]) @ W2[e] over the tokens routed to expert e,
computed with fp16 operands (fp32 PSUM accumulation).

Per-core layout: F (the 4096-wide hidden dim) is processed in NQ=4 quarters
with W1/W2 quarter-slices streamed through SBUF (double-buffered); the whole
fp16 xT stays resident. GEMM2 is computed transposed (stationary = W2 chunk,
moving = hT) so its PE cost scales with the exact token count instead of
quantizing to 128-row tiles, and y^T accumulates across quarters in SBUF via
DVE adds (no DRAM read-modify-write traffic, near-zero drain tail: each token
group's y is DMA'd out as soon as its last-quarter add lands).
"""

import numpy as np

B, S, H, E, F = 4, 2048, 1024, 8, 4096
T = B * S
P = 128
NQ = 4              # F quarters (outer loop); W1q + W2q streamed per quarter
FQ = F // NQ
KH = H // P         # k-chunks over H (GEMM1 contraction)
KFQ = FQ // P       # k-chunks over one F quarter (GEMM2 contraction)
HC = H // P         # output H chunks (GEMM2 transposed: psum partition = h')
TT = 512            # token group (matmul moving free dim)
MIN_CAP = 64

_cache = {}


def _spill_waits(nc, mybir, max_waits=1):
    """walrus CoreV2/V3 codegen rejects instructions with >1 semaphore wait
    ("Too many sync wait commands") — notably self-loading fp32/fp32r matmuls
    and DMACopy. Move excess waits onto same-engine no-ops inserted right
    before the instruction (sequencers run in order, so this is equivalent)."""
    for fn in nc.m.functions:
        for blk in fn.blocks:
            out = []
            changed = False
            for inst in blk.instructions:
                si = getattr(inst, "sync_info", None)
                if si is not None and len(si.on_wait) > max_waits:
                    spill = si.on_wait[: len(si.on_wait) - max_waits]
                    keep = si.on_wait[len(si.on_wait) - max_waits:]
                    for w in spill:
                        nop = mybir.InstNoOp(
                            name=nc.get_next_instruction_name(),
                            engine=inst.engine,
                            ins=[],
                            outs=[],
                        )
                        nop.sync_info = mybir.SyncInfo(on_wait=[w], on_update=[])
                        out.append(nop)
                    inst.sync_info = mybir.SyncInfo(on_wait=keep, on_update=si.on_update)
                    changed = True
                out.append(inst)
            if changed:
                blk.instructions = out


def _groups(cap):
    """Token groups: a small first group (so the startup DMA for it is tiny
    and the PE starts early), 512s after, and no tiny remainder group (tiny
    groups are LDWEIGHTS-bound: stream time < weight-load time)."""
    sizes = []
    o = 0
    while o < cap:
        tt = min(256 if not sizes else TT, cap - o)
        sizes.append(tt)
        o += tt
    if len(sizes) >= 2 and sizes[-1] < 280:
        pair = sizes[-2] + sizes[-1]
        sizes[-2] = -(-pair // 2)
        sizes[-1] = pair - sizes[-2]
    groups = []
    o = 0
    for tt in sizes:
        groups.append((o, tt))
        o += tt
    return groups


def _build(cap):
    import concourse.bass as bass
    import concourse.mybir as mybir
    from concourse import tile

    F32 = mybir.dt.float32
    # all matmul operands bf16: same PE speed as fp16, but the 7-bit mantissa
    # toggles far less of the multiplier array, which keeps the PE further
    # from its power-throttle ceiling (fp16 measurably throttles harder)
    SDT = mybir.dt.bfloat16
    GELU = mybir.ActivationFunctionType.Gelu_apprx_tanh
    ADD = mybir.AluOpType.add

    groups = _groups(cap)

    # all inputs are host-packed in exact SBUF layout, so every DMA row is
    # one long contiguous run per partition (DMA throughput is dominated by
    # per-row overhead; 1KB rows move ~3x slower than 8KB rows):
    #   xt: group-major, block g = [KH, tt] per partition
    #   w1: [q][fs][k][128] per partition (fs-major: the startup quarter
    #       streams in 8 fs-blocks, each immediately usable by one GEMM1
    #       accumulation chain)
    #   w2: [q][k2][H] per partition
    nc = bass.Bass()
    xt = nc.declare_dram_parameter("xt", [P, KH * cap], SDT, isOutput=False)
    w1 = nc.declare_dram_parameter("w1", [P, NQ * KFQ * KH * P], SDT, isOutput=False)
    w2 = nc.declare_dram_parameter("w2", [P, NQ * KFQ * H], SDT, isOutput=False)
    b1s = nc.declare_dram_parameter("b1s", [P, F // P], F32, isOutput=False)
    yt = nc.declare_dram_parameter("yt", [H, cap], F32, isOutput=True)

    with tile.TileContext(nc) as tc:
        with (
            tc.tile_pool(name="w1p", bufs=2) as w1p,
            tc.tile_pool(name="w2p", bufs=2) as w2p,
            tc.tile_pool(name="xp", bufs=1) as xp,
            tc.tile_pool(name="hp", bufs=1) as hp,
            tc.tile_pool(name="yp", bufs=1) as yp,
            tc.tile_pool(name="cst", bufs=1) as cst,
            tc.tile_pool(name="ps1", bufs=4, space="PSUM") as ps1,
            tc.tile_pool(name="ps2", bufs=4, space="PSUM") as ps2,
        ):
            def load_w1(q):
                # steady-state quarters: one DMA, 16KB contiguous rows
                w1q = w1p.tile([P, KFQ, KH, P], SDT, tag="w1q")
                nc.sync.dma_start(w1q[:], w1[:, q * QW1:(q + 1) * QW1])
                return w1q

            def load_w2(q, split=False):
                # DMA issues ride gpsimd/sync so the Scalar engine (which runs
                # the critical-path activations) never stalls on a descriptor
                w2q = w2p.tile([P, KFQ, H], SDT, tag="w2q")
                src = w2[:, q * QW2:(q + 1) * QW2]
                if split:
                    nc.gpsimd.dma_start(w2q[:, : KFQ // 2, :], src[:, : QW2 // 2])
                    nc.sync.dma_start(w2q[:, KFQ // 2:, :], src[:, QW2 // 2:])
                else:
                    nc.gpsimd.dma_start(w2q[:], src)
                return w2q

            QW1 = KFQ * KH * P
            QW2 = KFQ * H
            xoff = []
            o = 0
            for _, tt in groups:
                xoff.append(o)
                o += KH * tt

            # prologue. DMA throughput is row-overhead-bound, so feed order
            # matters more than anything: x(group 0) first on scalar, then
            # W1(quarter 0) as 8 fs-blocks alternating sync/gpsimd (each
            # immediately consumed by one GEMM1 chain), W2(quarter 0) halves
            # behind them, remaining x groups on scalar.
            b1t = cst.tile([P, F // P], F32)
            x_all = xp.tile([P, KH * cap], SDT)
            nc.scalar.dma_start(x_all[:, : KH * groups[0][1]],
                                xt[:, : KH * groups[0][1]])
            nc.scalar.dma_start(b1t[:], b1s[:])
            w1q = w1p.tile([P, KFQ, KH, P], SDT, tag="w1q")
            BL = KH * P
            for fs in range(KFQ):
                wq = nc.sync if fs % 2 == 0 else nc.gpsimd
                wq.dma_start(w1q[:, fs, :, :], w1[:, fs * BL:(fs + 1) * BL])
            w2q = load_w2(0, split=True)
            for gi in range(1, len(groups)):
                lo, hi = xoff[gi], xoff[gi] + KH * groups[gi][1]
                nc.scalar.dma_start(x_all[:, lo:hi], xt[:, lo:hi])
            yT = yp.tile([P, HC, cap], F32)
            ytdram = yt.rearrange("(c p) t -> p c t", p=P)

            w1qs = {0: w1q}
            w2qs = {0: w2q}
            hqs = {}

            def gemm1(q, gi, idx):
                t0, tt = groups[gi]
                xo = xoff[gi]
                w1q = w1qs[q]
                # hT[f, t] = sum_h W1[h, f] * xT[h, t], then gelu
                hq = hp.tile([P, KFQ, TT], SDT, tag=f"hq{idx % 2}")
                hqs[(q, gi)] = hq
                for fs in range(KFQ):
                    pt = ps1.tile([P, TT], F32, tag="pt1")
                    for k in range(KH):
                        nc.tensor.matmul(
                            pt[:, :tt],
                            w1q[:, fs, k, :],
                            x_all[:, xo + k * tt: xo + (k + 1) * tt],
                            start=(k == 0),
                            stop=(k == KH - 1),
                        )
                    c = q * KFQ + fs
                    nc.scalar.activation(
                        hq[:, fs, :tt], pt[:, :tt], GELU, bias=b1t[:, c:c + 1]
                    )
                if gi == 0 and q + 1 < NQ:
                    w1qs[q + 1] = load_w1(q + 1)
                if gi == 2 and q + 1 < NQ:
                    w2qs[q + 1] = load_w2(q + 1)

            def gemm2(q, gi):
                t0, tt = groups[gi]
                w2q = w2qs[q]
                hq = hqs.pop((q, gi))
                # transposed: yT[h', t] += sum_f W2[f, h'] * hT[f, t]
                for hc in range(HC):
                    pt2 = ps2.tile([P, TT], F32, tag="pt2")
                    for k2 in range(KFQ):
                        nc.tensor.matmul(
                            pt2[:, :tt],
                            w2q[:, k2, hc * P:(hc + 1) * P],
                            hq[:, k2, :tt],
                            start=(k2 == 0),
                            stop=(k2 == KFQ - 1),
                        )
                    if q == 0:
                        nc.vector.tensor_copy(yT[:, hc, t0:t0 + tt], pt2[:, :tt])
                    else:
                        nc.vector.tensor_tensor(
                            yT[:, hc, t0:t0 + tt],
                            yT[:, hc, t0:t0 + tt],
                            pt2[:, :tt],
                            ADD,
                        )
                    if q == NQ - 1:
                        # y^T for this (group, hc) is final: stream it out as
                        # soon as its add lands so almost nothing drains after
                        # the last matmul; alternate queues so a single
                        # backlogged queue can't delay the end
                        yq = nc.sync if hc % 2 == 0 else nc.gpsimd
                        yq.dma_start(
                            ytdram[:, hc, t0:t0 + tt], yT[:, hc, t0:t0 + tt]
                        )

            # software pipeline: GEMM1 runs one group ahead of GEMM2, so
            # GEMM2 never waits on the activations of its own group (the
            # per-group act-latency bubble disappears) and the PE has a full
            # group of GEMM1 work queued while startup DMAs stream in
            sched = [(q, gi) for q in range(NQ) for gi in range(len(groups))]
            for idx, (q, gi) in enumerate(sched):
                gemm1(q, gi, idx)
                if idx >= 1:
                    gemm2(*sched[idx - 1])
            gemm2(*sched[-1])

    import concourse.mybir as mybir_mod

    _spill_waits(nc, mybir_mod)
    return nc


def _route(x2d, Wr, br):
    """Top-2 routing, bit-matching the reference's decisions.

    Softmax is monotonic, so top-2-of-probs == top-2-of-logits, and the
    normalized top-1 weight p0 = p1/(p1+p2) == sigmoid(l1-l2) exactly (the
    softmax denominator cancels). Ordering ties are broken by lower index,
    same as jax.lax.top_k."""
    logits = x2d @ np.asarray(Wr, np.float32) + np.asarray(br, np.float32)
    order = np.argsort(-logits, axis=-1, kind="stable")
    i1 = order[:, 0].astype(np.int64)
    i2 = order[:, 1].astype(np.int64)
    r = np.arange(logits.shape[0])
    l1 = logits[r, i1].astype(np.float64)
    l2 = logits[r, i2].astype(np.float64)
    p0 = 1.0 / (1.0 + np.exp(l2 - l1))
    return i1, i2, p0.astype(np.float32)


def _plan(x, Wr, br, W1, b1, W2, b2):
    """Route on host; build the per-core (per-expert) input maps."""
    x2d = np.ascontiguousarray(np.asarray(x, np.float32).reshape(T, H))
    W1 = np.asarray(W1, np.float32)
    b1 = np.asarray(b1, np.float32)
    W2 = np.asarray(W2, np.float32)

    i1, i2, p0 = _route(x2d, Wr, br)
    idxs = [np.flatnonzero((i1 == e) | (i2 == e)) for e in range(E)]
    max_cnt = max(len(ix) for ix in idxs)
    cap = max(MIN_CAP, -(-max_cnt // 2) * 2)

    import ml_dtypes

    BF16 = ml_dtypes.bfloat16
    groups = _groups(cap)
    xT = np.ascontiguousarray(x2d.T)  # [H, T]
    in_maps = []
    for e in range(E):
        ix = idxs[e]
        xte = np.zeros((H, cap), np.float32)
        xte[:, : len(ix)] = xT[:, ix]
        # pack everything in exact SBUF layout (see _build)
        xr = xte.reshape(KH, P, cap)
        xpk = np.concatenate(
            [
                xr[:, :, t0:t0 + tt].transpose(1, 0, 2).reshape(P, KH * tt)
                for t0, tt in groups
            ],
            axis=1,
        )
        w1pk = (
            W1[e]
            .reshape(KH, P, NQ, KFQ, P)
            .transpose(1, 2, 3, 0, 4)
            .reshape(P, NQ * KFQ * KH * P)
        )
        w2pk = (
            W2[e]
            .reshape(NQ, KFQ, P, H)
            .transpose(2, 0, 1, 3)
            .reshape(P, NQ * KFQ * H)
        )
        b1se = np.ascontiguousarray(b1[e].reshape(F // P, P).T)
        in_maps.append(
            {
                "xt": np.ascontiguousarray(xpk).astype(BF16),
                "w1": np.ascontiguousarray(w1pk).astype(BF16),
                "w2": np.ascontiguousarray(w2pk).astype(BF16),
                "b1s": b1se,
            }
        )
    return cap, in_maps, idxs, p0


def kernel(x, Wr, br, W1, b1, W2, b2):
    from concourse.bass_utils import run_bass_kernel_spmd

    cap, in_maps, idxs, p0 = _plan(x, Wr, br, W1, b1, W2, b2)

    if cap not in _cache:
        _cache[cap] = _build(cap)
    nc = _cache[cap]

    try:
        res = run_bass_kernel_spmd(nc, in_maps, list(range(E)))
    except Exception:
        import time as _time

        _time.sleep(10)
        res = run_bass_kernel_spmd(nc, in_maps, list(range(E)))

    b2 = np.asarray(b2, np.float32)
    out = np.zeros((T, H), np.float32)
    for e in range(E):
        ix = idxs[e]
        ye = res.results[e]["yt"][:, : len(ix)].T  # [cnt, H]
        out[ix] += p0[ix, None] * (ye + b2[e][None, :])
    return out.reshape(B, S, H)
